# revision 57
# baseline (speedup 1.0000x reference)
"""Trainium2 Bass kernel: AttentionWithFeedForward (dense transformer block).

Sharding: 8 cores = (batch b = c//4) x (seq chunk of 1024 tokens = c%4).
Each core redundantly computes K/V over its full batch (no collectives),
Q/attention/FFN only for its own 1024-token chunk. The host rotates the
token axis per core so the own chunk is always columns 0:1024 (attention
is invariant to key order), keeping the device program identical across
cores.

Layout: all activations transposed [d_model, tok] ("ptile" layout
[128, d/128, tok]); host pre-transposes x/y, pre-casts weights, and folds
the V-projection biases into the attention out-projection biases
(softmax weights sum to 1, so  (AV/den + vb) @ Wo + bo  ==
AV/den @ Wo + (vb @ Wo + bo)).

Perf structure (~590us vs the 702us baseline; per-core roofline is the
softmax exp: 33.5M scores / (128 lanes x 1.2GHz ScalarE) ~ 250us):
- QKV projection and both attention out-projections run as fp8e4
  DoubleRow matmuls (2 contraction k-tiles per instruction -> half the
  PE column-streams). x / w_qkv / w_sao / w_cao are fp8 from the host;
  attention internals (q/k/v/et, CA k/v/y) are fp8 end-to-end; the
  residual stream stays f32/bf16.
- Self-attention runs (head-pair, n-slice)-outer passes: PSUM =
  3 x scores[128,2,512] (6 banks) + 2 x 1-bank AV accumulators = 8.
  Triple-buffered scores + a 2-unit software-pipeline skew on the AV
  matmuls keep TensorE streaming while ScalarE exp paces the loop at
  ~1us per (pair, n, kk) unit.
- Token-slice pipelining: the network is token-parallel outside
  attention, so slice-0's tail (out-proj -> LN1 -> CA -> LN2) is
  emission-interleaved into slice-1's ScalarE-bound SA passes, and
  slice-0's FFN overlaps slice-1's tail. LN3(slice 0) runs inside the
  CA(slice 1) denominator-flush latency hole.
- Attention denominators: quick PSUM->SBUF copies release the AV
  accumulators in ~1us; 1/den is DVE `reciprocal` (3.6us/row, the only
  HW-supported divide), batched 8-at-a-time for the deferred passes
  (cost is free-size-based, so one [8,512] recip == one row).
  Broadcasts go through a DRAM round-trip (step-0 partition DMA) --
  gpsimd.partition_broadcast mis-executes on HW and DVE lane-locking
  forbids partition moves.
- No Ln in the hot loop -> no 1.3us ACT_TABLE_LOAD swaps (Exp, Square,
  Copy, Identity share every table; Sqrt/Gelu swap only at phase edges).
- LN per slice: stats sums via ones-column matmuls into one sc-tile
  (never the oa/ob rings -- ring reuse serializes unrelated phases),
  mean/rstd broadcast via K=1 fp32 ones-matmul, biases applied as
  ScalarE Identity activations to keep DVE free.
- All engine partition ranges stay in 32-aligned blocks (DVE/ACT
  hardware requirement; access from base 32 may span at most 32 rows).

Softmax without max subtraction (scores are tightly bounded at this
problem's scale); denominators come from a ones-column packed into V
([Ve(64) | ones(1) | pad(31) | Vo(64)] = 160 cols per head pair): even
heads read AV rows 0..64 with the denominator at row 64; odd heads use a
128-wide shifted view (cols 32..160) so their output lands on partitions
64..127 with the denominator at row 32 (DVE partition bases must be
32-aligned, which rules out tighter packings).

SBUF is a two-sided stack allocator: frees must be LIFO per side.
"""

from contextlib import ExitStack

import numpy as np
import ml_dtypes

import concourse.bass as bass
import concourse.tile as tile
from concourse import bacc, mybir
from concourse.bass_utils import run_bass_kernel_spmd

BF16 = mybir.dt.bfloat16
F32 = mybir.dt.float32
FP8 = mybir.dt.float8e4
AF = mybir.ActivationFunctionType
OP = mybir.AluOpType
DR = mybir.MatmulPerfMode.DoubleRow

P = 128
D = 512          # d_embed
EJ = D // P      # 4 ptiles
DC = 768         # d_cross
CJ = DC // P     # 6
FF = 2048
FJ = FF // P     # 16
H = 8
DH = 64
S = 4096
ST = S // P      # 32 key tiles (full batch)
CH = 1024        # tokens per core
N2 = CH // 512   # 2 free-dim slices
B = 2
NCORES = 8
EPS = 1e-5
GELU_AF = AF.Gelu_apprx_tanh
USE_GPB = False     # gpsimd partition_broadcast vs DMA round-trip
LN_MM_BCAST = True  # LN mean/rstd broadcast via K=1 fp32 matmul vs DMA

# bias_cols column layout; column j of a param holds param[128*j + p].
_BC = {}
_c = 0
for _nm, _n in [("qb", 4), ("kb", 4), ("saob", 4), ("caqb", 4),
                ("cakb", 4), ("caob", 4), ("ffb1", 16),
                ("ffb2", 4), ("ln1g", 4), ("ln1b", 4), ("ln2g", 4),
                ("ln2b", 4), ("ln3g", 4), ("ln3b", 4)]:
    _BC[_nm] = (_c, _n)
    _c += _n
NBC = _c


def _pt(a):
    """[din, N] -> [128, din//128, N] ptile layout (partition-inner)."""
    din, n = a.shape
    return np.ascontiguousarray(a.reshape(din // P, P, n).transpose(1, 0, 2))


def _bcol(v):
    """[din] -> [128, din//128]."""
    return np.ascontiguousarray(v.reshape(-1, P).T)


def _bcast_ap(row_ap, nparts):
    """Broadcast a [1, N] DRAM AP across nparts partitions (step 0)."""
    return bass.AP(tensor=row_ap.tensor, offset=row_ap.offset,
                   ap=[[0, nparts]] + [list(d) for d in row_ap.ap[1:]])


def build(ctx, tc, dram):
    """Emit the full per-core program. Returns (names, out_name)."""
    nc = tc.nc
    names = {}

    def din(key, shape, dtype):
        t = dram.tile(shape, dtype, kind="ExternalInput", name=f"i_{key}")
        names[key] = t.name
        return t

    # ---- DRAM I/O ----
    xt8_d = din("xt8", [P, EJ, S], FP8)          # x[b].T rotated, fp8
    xt_f32_d = din("xt_f32", [P, EJ, CH], BF16)  # own chunk (cols 0:CH), bf16
    yt_d = din("yt", [P, CJ, 77], FP8)          # y[b].T
    w_qkv_d = din("w_qkv", [P, EJ, 3 * D], FP8)
    w_sao_d = din("w_sao", [P, EJ, D], FP8)
    w_caq_d = din("w_caq", [P, EJ, D], BF16)
    w_cak_d = din("w_cak", [P, CJ, D], FP8)
    w_cav_d = din("w_cav", [P, CJ, D], FP8)
    w_cao_d = din("w_cao", [P, EJ, D], FP8)
    w_ff1_d = din("w_ff1", [P, EJ, FF], BF16)
    w_ff2_d = din("w_ff2", [P, FJ, D], BF16)
    bias_d = din("bias", [P, NBC], F32)
    out_d = dram.tile([P, EJ, CH], F32, kind="ExternalOutput", name="o_out")
    out_name = out_d.name

    dma = nc.sync.dma_start

    def sb(key, shape, dtype, side):
        return tc.tile(shape, dtype, name=f"s_{key}", side=side)

    # ---- pools ----
    # PSUM: sc 3x2 banks + oa 1 + ob 1 = 8 banks exactly.
    ps = ctx.enter_context(tc.tile_pool(name="ps", bufs=1, space="PSUM"))
    et_pool = ctx.enter_context(
        tc.tile_pool(name="et_pool", bufs=4, side="left"))
    osb_pool = ctx.enter_context(
        tc.tile_pool(name="osb_pool", bufs=9, side="left"))
    rep_pool = ctx.enter_context(
        tc.tile_pool(name="rep_pool", bufs=2, side="left"))
    lnq_pool = ctx.enter_context(
        tc.tile_pool(name="lnq_pool", bufs=2, side="left"))
    res_pool = ctx.enter_context(
        tc.tile_pool(name="res_pool", bufs=2, side="left"))
    hbf_pool = ctx.enter_context(
        tc.tile_pool(name="hbf_pool", bufs=1, side="left"))
    dsc_pool = ctx.enter_context(
        tc.tile_pool(name="dsc_pool", bufs=8, space="DRAM"))

    def bcast(dst_rows, src_row, n):
        """Broadcast src_row [1, n-free] to all partitions of dst_rows."""
        if USE_GPB:
            nc.gpsimd.partition_broadcast(dst_rows, src_row)
        else:
            dsc = dsc_pool.tile([1, n], F32, tag="dsc", name="dsc")
            dma(out=dsc[0:1, :], in_=src_row.rearrange("p ... -> p (...)"))
            dma(out=dst_rows.rearrange("p ... -> p (...)"),
                in_=_bcast_ap(dsc[0:1, :], dst_rows.shape[0]))

    def sc_tile(name):
        return ps.tile([P, 2, 512], F32, tag="sc", bufs=3, name=name)

    # ---- permanent small tiles (right-side bottom) ----
    bias_t, free_bias = sb("bias", [P, NBC], F32, "right")

    def bc(nm, j):
        c0, _n = _BC[nm]
        return bias_t[:, c0 + j:c0 + j + 1]

    ones_col, free_ones = sb("ones_col", [P, 1], BF16, "right")
    nc.vector.memset(ones_col[:, :], 1.0)
    ones_row, free_ones_row = sb("ones_row", [1, P], F32, "right")
    nc.vector.memset(ones_row[:, :], 1.0)
    yt, free_yt = sb("yt", [P, CJ, 77], FP8, "right")
    dma(out=yt[:, :, :], in_=yt_d[:, :, :])

    # small long-lived CA tensors + prefetched weights (right-side)
    kc, free_kc = sb("kc", [P, EJ, 77], FP8, "right")
    vc1, free_vc1 = sb("vc1", [77, 1, (H // 2) * 160], FP8, "right")
    w_sao, free_w_sao = sb("w_sao", [P, EJ, D], FP8, "right")
    dma(out=w_sao[:, :, :], in_=w_sao_d[:, :, :])
    w_caq, free_w_caq = sb("w_caq", [P, EJ, D], BF16, "right")
    dma(out=w_caq[:, :, :], in_=w_caq_d[:, :, :])
    w_cao, free_w_cao = sb("w_cao", [P, EJ, D], FP8, "right")
    dma(out=w_cao[:, :, :], in_=w_cao_d[:, :, :])
    w_ff1, free_w_ff1 = sb("w_ff1", [P, EJ, FF], BF16, "right")
    dma(out=w_ff1[:, :, :], in_=w_ff1_d[:, :, :])
    # ---- left stack: SA-phase tensors (LIFO: frees before SA end go on top)
    xt_f32, free_xt_f32 = sb("xt_f32", [P, EJ, CH], BF16, "left")
    dma(out=xt_f32[:, :, :], in_=xt_f32_d[:, :, :])
    ot, free_ot = sb("ot", [P, EJ, CH], FP8, "left")
    qt, free_qt = sb("qt", [P, EJ, CH], FP8, "left")
    kt, free_kt = sb("kt", [P, EJ, S], FP8, "left")
    v1, free_v1 = sb("v1", [P, ST, (H // 2) * 160], FP8, "left")
    # freed right after the projection phase:
    xt8, free_xt8 = sb("xt8", [P, EJ, S], FP8, "left")
    w_qkv, free_w_qkv = sb("w_qkv", [P, EJ, 3 * D], FP8, "left")
    dma(out=w_qkv[:, :, 0:D], in_=w_qkv_d[:, :, 0:D])
    dma(out=xt8[:, :, 0:CH], in_=xt8_d[:, :, 0:CH])
    dma(out=bias_t[:, :], in_=bias_d[:, :])
    dma(out=xt8[:, :, CH:S], in_=xt8_d[:, :, CH:S])
    dma(out=w_qkv[:, :, D:3 * D], in_=w_qkv_d[:, :, D:3 * D])
    w_cak, free_w_cak = sb("w_cak", [P, CJ, D], FP8, "left")
    dma(out=w_cak[:, :, :], in_=w_cak_d[:, :, :])
    w_cav, free_w_cav = sb("w_cav", [P, CJ, D], FP8, "left")
    dma(out=w_cav[:, :, :], in_=w_cav_d[:, :, :])

    v1h = v1[:, :, :].rearrange("p t (pr c) -> p t pr c", c=160)
    nc.vector.memset(v1h[:, :, :, 64:65], 1.0)
    nc.vector.memset(v1h[:, :, :, 65:96], 0.0)

    # ---- phase 1: QKV projections, fp8 DoubleRow over e-pairs ----
    def proj_dr(w_t, c0, j, rhs_t, n0):
        """psum[:, n, :] over n in 0,1 = w[:, :, c0+128j]T. @ rhs cols
        [512*(n0+n)]; DoubleRow contract over both e-pairs."""
        p_t = sc_tile("ps_dr")
        for ep in range(2):
            for n in range(2):
                nc.tensor.matmul(
                    p_t[:, n, :],
                    lhsT=w_t[:, 2 * ep:2 * ep + 2,
                             c0 + P * j:c0 + P * (j + 1)],
                    rhs=rhs_t[:, 2 * ep:2 * ep + 2,
                              512 * (n0 + n):512 * (n0 + n + 1)],
                    start=(ep == 0), stop=(ep == 1), perf_mode=DR)
        return p_t

    for j in range(EJ):                      # Q (own chunk only)
        p_t = proj_dr(w_qkv, 0, j, xt8, 0)
        nc.scalar.activation(
            qt[:, j, :].rearrange("p (a b) -> p a b", b=512),
            p_t[:, :, :], AF.Identity, bias=bc("qb", j))
    for j in range(EJ):                      # K (full batch)
        for nn in range(S // CH):
            p_t = proj_dr(w_qkv, D, j, xt8, 2 * nn)
            nc.scalar.activation(
                kt[:, j, CH * nn:CH * (nn + 1)].rearrange(
                    "p (a b) -> p a b", b=512),
                p_t[:, :, :], AF.Identity, bias=bc("kb", j))
    for tp in range(ST // 2):                # V (full batch), token-major
        p_t = sc_tile("ps_v")
        for t2 in range(2):
            t = 2 * tp + t2
            for ep in range(2):
                nc.tensor.matmul(
                    p_t[:, t2, :],
                    lhsT=xt8[:, 2 * ep:2 * ep + 2, P * t:P * (t + 1)],
                    rhs=w_qkv[:, 2 * ep:2 * ep + 2, 2 * D:3 * D],
                    start=(ep == 0), stop=(ep == 1), perf_mode=DR)
        for t2 in range(2):
            t = 2 * tp + t2
            psh = p_t[:, t2, :].rearrange("p (pr two c) -> p pr two c",
                                          two=2, c=64)
            # V bias folded into out-proj bias on host; none applied here.
            nc.scalar.activation(v1h[:, t, :, 0:64], psh[:, :, 0, :], AF.Copy)
            nc.scalar.activation(v1h[:, t, :, 96:160], psh[:, :, 1, :],
                                 AF.Copy)

    # ---- phase 1b: CA K/V from y (independent of x; fills PE bubbles) ----
    vc1h = vc1[:, :, :].rearrange("p t (pr c) -> p t pr c", c=160)
    nc.vector.memset(vc1h[:, :, :, 64:65], 1.0)
    nc.vector.memset(vc1h[:, :, :, 65:96], 0.0)
    for j in range(EJ):
        p_t = sc_tile("ps_ck")
        for e in range(CJ):
            nc.tensor.matmul(p_t[:, 0, 0:77],
                             lhsT=w_cak[:, e, P * j:P * (j + 1)],
                             rhs=yt[:, e, :],
                             start=(e == 0), stop=(e == CJ - 1))
        nc.vector.tensor_scalar(out=kc[:, j, :], in0=p_t[:, 0, 0:77],
                                scalar1=bc("cakb", j), scalar2=None,
                                op0=OP.add)
    psv = sc_tile("ps_cv")
    for e in range(CJ):
        nc.tensor.matmul(psv[0:77, 0, :], lhsT=yt[:, e, :],
                         rhs=w_cav[:, e, :], start=(e == 0),
                         stop=(e == CJ - 1))
    psvh = psv[0:77, 0, :].rearrange("p (pr two c) -> p pr two c", two=2, c=64)
    nc.vector.tensor_copy(out=vc1h[:, 0, :, 0:64], in_=psvh[:, :, 0, :])
    nc.vector.tensor_copy(out=vc1h[:, 0, :, 96:160], in_=psvh[:, :, 1, :])

    free_w_cav()
    free_w_cak()
    free_w_qkv()
    free_xt8()
    w_ff2, free_w_ff2 = sb("w_ff2", [P, FJ, D], BF16, "right")
    dma(out=w_ff2[:, :, :], in_=w_ff2_d[:, :, :])

    # ---- attention core: one (head-pair, n-slice) pass ----
    def attn_pass(jp, n, kv_tiles, kp, kt_t, qt_t, v1_t, out_t,
                  defer=None):
        """Accumulate both heads of pair jp over all key tiles for query
        slice n, then normalize into out_t (fp8).  oa: even head, value
        rows 0..63 + denominator at row 64.  ob: odd head at rows 64..127
        + denominator at row 32 (shifted [V|1] view)."""
        oa = ps.tile([65, 512], F32, tag="oa", bufs=1, name="oa")
        ob = ps.tile([P, 512], F32, tag="ob", bufs=1, name="ob")
        q_sl = slice(512 * n, 512 * (n + 1))
        pend = []          # (kk, et) awaiting AV emission (2-unit skew)

        def emit_av(kk, et_t):
            nc.tensor.matmul(oa[:, :],
                             lhsT=v1_t[0:kp, kk, 160 * jp:160 * jp + 65],
                             rhs=et_t[0:kp, 0, :],
                             start=(kk == 0), stop=(kk == kv_tiles - 1))
            nc.tensor.matmul(ob[:, :],
                             lhsT=v1_t[0:kp, kk, 160 * jp + 32:160 * jp + 160],
                             rhs=et_t[0:kp, 1, :],
                             start=(kk == 0), stop=(kk == kv_tiles - 1))

        for kk in range(kv_tiles):
            sc = sc_tile("sc")
            nc.tensor.matmul(sc[0:kp, 0, :],
                             lhsT=kt_t[0:DH, jp, P * kk:P * kk + kp],
                             rhs=qt_t[0:DH, jp, q_sl],
                             start=True, stop=True)
            nc.tensor.matmul(sc[0:kp, 1, :],
                             lhsT=kt_t[DH:P, jp, P * kk:P * kk + kp],
                             rhs=qt_t[DH:P, jp, q_sl],
                             start=True, stop=True)
            et = et_pool.tile([P, 2, 512], FP8, tag="et", name="et")
            nc.scalar.activation(et[0:kp, :, :], sc[0:kp, :, :], AF.Exp,
                                 scale=0.125)
            pend.append((kk, et))
            if len(pend) > 2:
                emit_av(*pend.pop(0))
        for item in pend:
            emit_av(*item)

        # normalize: two quick copies (value rows + den row, each within an
        # aligned DVE partition block) release the accumulators for the next
        # pass within ~1us; the 1/den multiply runs off-loop from SBUF.
        for odd, o_ps, d_row in ((0, oa, 64), (1, ob, 32)):
            orng = slice(64, 128) if odd else slice(0, 64)
            osb = osb_pool.tile([P, 512], F32, tag="osb", name="osb")
            nc.vector.tensor_copy(out=osb[orng, :], in_=o_ps[orng, :])
            nc.vector.tensor_copy(out=osb[d_row:d_row + 1, :],
                                  in_=o_ps[d_row:d_row + 1, :])
            if defer is not None:
                defer.append((osb, d_row, orng, jp, q_sl))
                continue
            nc.vector.reciprocal(osb[d_row:d_row + 1, :],
                                 osb[d_row:d_row + 1, :])
            rep = rep_pool.tile([P, 512], F32, tag="rep", name="rep")
            bcast(rep[:, :], osb[d_row:d_row + 1, :], 512)
            nc.vector.tensor_tensor(out=out_t[orng, jp, q_sl],
                                    in0=osb[orng, :],
                                    in1=rep[orng, :], op=OP.mult)

    # ---- slice-parameterized tail helpers ----
    x1, free_x1 = sb("x1", [P, EJ, CH], BF16, "right")
    qc, free_qc = sb("qc", [P, EJ, CH], FP8, "right")
    oct_, free_oct = sb("oct", [P, EJ, CH], FP8, "right")
    x2, free_x2 = sb("x2", [P, EJ, CH], BF16, "right")

    def proj_resid_dr_sl(w_t, in_t, res_t, dst_t, b_nm, n):
        """dst (f32, [P,EJ,512]) = w.T @ in[:, :, q_sl] (fp8 DR) + bias
        + res[:, :, q_sl]."""
        q_sl = slice(512 * n, 512 * (n + 1))
        for j2 in range(2):
            p_t = sc_tile("ps_pr")
            for jj in range(2):
                j = 2 * j2 + jj
                for ep in range(2):
                    nc.tensor.matmul(
                        p_t[:, jj, :],
                        lhsT=w_t[:, 2 * ep:2 * ep + 2, P * j:P * (j + 1)],
                        rhs=in_t[:, 2 * ep:2 * ep + 2, q_sl],
                        start=(ep == 0), stop=(ep == 1), perf_mode=DR)
            for jj in range(2):
                j = 2 * j2 + jj
                nc.scalar.activation(dst_t[:, j, :], p_t[:, jj, :],
                                     AF.Identity, bias=bc(b_nm, j))
                nc.vector.tensor_tensor(out=dst_t[:, j, :],
                                        in0=dst_t[:, j, :],
                                        in1=res_t[:, j, q_sl], op=OP.add)

    def layernorm_sl(src_t, dst_t, dst_sl, g_nm, b_nm, on_act, out_dma=None):
        """LN over d for one 512-token slice. src_t [P,EJ,512] f32
        (destroyed); writes dst_t[:, j, dst_sl]."""
        xq = lnq_pool.tile([P, EJ, 512], BF16, tag="lnq", name="xq")
        sq = lnq_pool.tile([P, EJ, 512], BF16, tag="lnq", name="sq")
        if on_act:
            nc.scalar.activation(xq[:, :, :], src_t[:, :, :], AF.Copy)
            nc.scalar.activation(sq[:, :, :], xq[:, :, :], AF.Square)
        else:
            nc.vector.tensor_copy(out=xq[:, :, :], in_=src_t[:, :, :])
            nc.vector.tensor_tensor(out=sq[:, :, :], in0=xq[:, :, :],
                                    in1=xq[:, :, :], op=OP.mult)
        sums = sc_tile("ln_sums")
        for slot, srct in ((0, xq), (1, sq)):
            for e in range(EJ):
                nc.tensor.matmul(sums[0:1, slot, :], lhsT=ones_col[:, :],
                                 rhs=srct[:, e, :],
                                 start=(e == 0), stop=(e == EJ - 1))
        # st slots: 0 = mean, 1 = var+eps, 2 = mean^2 -> sigma -> rstd
        st = lnq_pool.tile([1, 3, 512], F32, tag="lnst", name="st")
        nc.vector.tensor_scalar(out=st[0:1, 0, :], in0=sums[0:1, 0, :],
                                scalar1=1.0 / D, scalar2=None, op0=OP.mult)
        nc.vector.tensor_scalar(out=st[0:1, 1, :], in0=sums[0:1, 1, :],
                                scalar1=1.0 / D, scalar2=EPS, op0=OP.mult,
                                op1=OP.add)
        nc.vector.tensor_tensor(out=st[0:1, 2, :], in0=st[0:1, 0, :],
                                in1=st[0:1, 0, :], op=OP.mult)
        nc.vector.tensor_tensor(out=st[0:1, 1, :], in0=st[0:1, 1, :],
                                in1=st[0:1, 2, :], op=OP.subtract)
        nc.scalar.activation(st[0:1, 2, :], st[0:1, 1, :], AF.Sqrt)
        nc.vector.reciprocal(st[0:1, 2, :], st[0:1, 2, :])
        # mean/rstd -> all partitions: K=1 fp32 matmul, copied out of PSUM
        # immediately so the shared sc ring is held only briefly.
        rp = sc_tile("rp_ln")
        for sl_, slot in ((0, 0), (2, 1)):
            nc.tensor.matmul(rp[:, slot, :], lhsT=ones_row[0:1, :],
                             rhs=st[0:1, sl_, :], start=True, stop=True)
        ostg = None
        if out_dma is not None:
            ostg = lnq_pool.tile([P, EJ, 512], F32, tag="ostg", bufs=1,
                                 name="ostg")
        rep_m = rep_pool.tile([P, 512], F32, tag="repln", bufs=3,
                              name="rep_m")
        rep_r = rep_pool.tile([P, 512], F32, tag="repln", bufs=3,
                              name="rep_r")
        nc.vector.tensor_copy(out=rep_m[:, :], in_=rp[:, 0, :])
        nc.vector.tensor_copy(out=rep_r[:, :], in_=rp[:, 1, :])
        for j in range(EJ):
            xv = src_t[:, j, :]
            nc.vector.tensor_tensor(out=xv, in0=xv, in1=rep_m[:, :],
                                    op=OP.subtract)
            nc.vector.tensor_tensor(out=xv, in0=xv, in1=rep_r[:, :],
                                    op=OP.mult)
            if out_dma is not None:
                nc.vector.tensor_scalar(out=ostg[:, j, :], in0=xv,
                                        scalar1=bc(g_nm, j),
                                        scalar2=bc(b_nm, j),
                                        op0=OP.mult, op1=OP.add)
                dma(out=out_d[:, j, 512 * out_dma:512 * (out_dma + 1)],
                    in_=ostg[:, j, :])
            else:
                nc.vector.tensor_scalar(out=dst_t[:, j, dst_sl], in0=xv,
                                        scalar1=bc(g_nm, j),
                                        scalar2=bc(b_nm, j),
                                        op0=OP.mult, op1=OP.add)

    def qc_sl(n):
        q_sl = slice(512 * n, 512 * (n + 1))
        for j2 in range(2):
            p_t = sc_tile("ps_cq")
            for jj in range(2):
                j = 2 * j2 + jj
                for e in range(EJ):
                    nc.tensor.matmul(
                        p_t[:, jj, :], lhsT=w_caq[:, e, P * j:P * (j + 1)],
                        rhs=x1[:, e, q_sl],
                        start=(e == 0), stop=(e == EJ - 1))
            for jj in range(2):
                j = 2 * j2 + jj
                nc.scalar.activation(qc[:, j, q_sl], p_t[:, jj, :],
                                     AF.Identity, bias=bc("caqb", j))

    def flush_norms(items, out_t):
        # batched denominators: one gather DMA -> one reciprocal -> scatter
        nrm = len(items)
        dend = dsc_pool.tile([nrm, 512], F32, tag="dend", name="dend")
        for i, (osb, d_row, _orng, _jp, _q) in enumerate(items):
            dma(out=dend[i:i + 1, :], in_=osb[d_row:d_row + 1, :])
        denr = osb_pool.tile([nrm, 512], F32, tag="denr", bufs=2, name="denr")
        dma(out=denr[:, :], in_=dend[:, :])
        nc.vector.reciprocal(denr[:, :], denr[:, :])
        dres = dsc_pool.tile([nrm, 512], F32, tag="dend", name="dres")
        dma(out=dres[:, :], in_=denr[:, :])
        for i, (osb, _d, orng, jp, q_sl) in enumerate(items):
            rep = rep_pool.tile([P, 512], F32, tag="rep", name="rep")
            dma(out=rep[orng, :], in_=_bcast_ap(dres[i:i + 1, :], 64))
            nc.vector.tensor_tensor(out=out_t[orng, jp, q_sl],
                                    in0=osb[orng, :],
                                    in1=rep[orng, :], op=OP.mult)
        items.clear()

    def ca_sl(n, flush_every=4):
        items = []
        for jp in range(H // 2):
            attn_pass(jp, n, 1, 77, kc, qc, vc1, oct_, defer=items)
            if flush_every and (jp + 1) % flush_every == 0:
                flush_norms(items, oct_)
        return items

    def ff1_sl(n, hbf, fp_range):
        q_sl = slice(512 * n, 512 * (n + 1))
        for fp in fp_range:
            p_t = sc_tile("ps_f1")
            for f2 in range(2):
                f = 2 * fp + f2
                for e in range(EJ):
                    nc.tensor.matmul(
                        p_t[:, f2, :], lhsT=w_ff1[:, e, P * f:P * (f + 1)],
                        rhs=x2[:, e, q_sl],
                        start=(e == 0), stop=(e == EJ - 1))
            for f2 in range(2):
                f = 2 * fp + f2
                nc.scalar.activation(hbf[:, f, :], p_t[:, f2, :],
                                     GELU_AF, bias=bc("ffb1", f))

    def ff2_sl(n, hbf, dst_t):
        q_sl = slice(512 * n, 512 * (n + 1))
        for j2 in range(2):
            p_t = sc_tile("ps_f2")
            for jj in range(2):
                j = 2 * j2 + jj
                for f in range(FJ):
                    nc.tensor.matmul(
                        p_t[:, jj, :], lhsT=w_ff2[:, f, P * j:P * (j + 1)],
                        rhs=hbf[:, f, :],
                        start=(f == 0), stop=(f == FJ - 1))
            for jj in range(2):
                j = 2 * j2 + jj
                nc.scalar.activation(dst_t[:, j, :], p_t[:, jj, :],
                                     AF.Identity, bias=bc("ffb2", j))
                nc.vector.tensor_tensor(out=dst_t[:, j, :],
                                        in0=dst_t[:, j, :],
                                        in1=x2[:, j, q_sl], op=OP.add)

    def res_tile(nm):
        return res_pool.tile([P, EJ, 512], BF16, tag="res", name=nm)

    S0, S1 = slice(0, 512), slice(512, 1024)

    # ---- phase 2: self-attention slice 0, then slice 1 with the slice-0
    # tail (out-proj, LN1, CA, LN2) interleaved into its ScalarE-bound span
    for jp in range(H // 2):
        attn_pass(jp, 0, ST, P, kt, qt, v1, ot)

    sa1 = []
    attn_pass(0, 1, ST, P, kt, qt, v1, ot, defer=sa1)
    r0 = res_tile("r0")
    proj_resid_dr_sl(w_sao, ot, xt_f32, r0, "saob", 0)
    attn_pass(1, 1, ST, P, kt, qt, v1, ot, defer=sa1)
    layernorm_sl(r0, x1, S0, "ln1g", "ln1b", on_act=False)
    attn_pass(2, 1, ST, P, kt, qt, v1, ot, defer=sa1)
    qc_sl(0)
    flush_norms(sa1, ot)
    ca_sl(0)
    attn_pass(3, 1, ST, P, kt, qt, v1, ot, defer=sa1)
    r1 = res_tile("r1")
    proj_resid_dr_sl(w_cao, oct_, x1, r1, "caob", 0)
    layernorm_sl(r1, x2, S0, "ln2g", "ln2b", on_act=False)
    flush_norms(sa1, ot)

    # ---- post-SA: slice-0 FFN overlapped with the slice-1 tail ----
    free_v1()
    free_kt()
    free_qt()
    hb0 = hbf_pool.tile([P, FJ, 512], BF16, tag="hbf", name="hb0")
    ff1_sl(0, hb0, range(0, 4))
    r2 = res_tile("r2")
    proj_resid_dr_sl(w_sao, ot, xt_f32, r2, "saob", 1)
    free_ot()
    free_xt_f32()
    ff1_sl(0, hb0, range(4, 8))
    layernorm_sl(r2, x1, S1, "ln1g", "ln1b", on_act=True)
    qc_sl(1)
    r3 = res_tile("r3")
    ff2_sl(0, hb0, r3)
    ca1_items = ca_sl(1, flush_every=0)
    layernorm_sl(r3, r3, S0, "ln3g", "ln3b", on_act=True, out_dma=0)
    flush_norms(ca1_items, oct_)
    r4 = res_tile("r4")
    proj_resid_dr_sl(w_cao, oct_, x1, r4, "caob", 1)
    layernorm_sl(r4, x2, S1, "ln2g", "ln2b", on_act=True)
    hb1 = hbf_pool.tile([P, FJ, 512], BF16, tag="hbf", name="hb1")
    ff1_sl(1, hb1, range(0, 8))
    r5 = res_tile("r5")
    ff2_sl(1, hb1, r5)
    layernorm_sl(r5, r5, S0, "ln3g", "ln3b", on_act=True, out_dma=1)

    free_x2()
    free_oct()
    free_qc()
    free_x1()
    free_w_ff2()
    free_w_ff1()
    free_w_cao()
    free_w_caq()
    free_w_sao()
    free_vc1()
    free_kc()
    free_yt()
    free_ones_row()
    free_ones()
    free_bias()

    return names, out_name


_CACHE = {}


def _compiled():
    if "nc" not in _CACHE:
        nc = bacc.Bacc("TRN2", target_bir_lowering=False, debug=False)
        with tile.TileContext(nc) as tc:
            with tc.tile_pool(name="dram_io", bufs=1, space="DRAM") as dram:
                with ExitStack() as ctx:
                    names, out_name = build(ctx, tc, dram)
        nc.compile()
        _CACHE["nc"] = (nc, names, out_name)
    return _CACHE["nc"]


def make_in_maps(inputs, names):
    """Host-side sharding: full inputs -> 8 per-core in_maps."""
    bf = ml_dtypes.bfloat16
    f8 = ml_dtypes.float8_e4m3
    f32 = np.float32
    x = np.asarray(inputs["x"], f32)
    y = np.asarray(inputs["y"], f32)
    w = {k: np.asarray(v, f32) for k, v in inputs.items()}

    # fold V biases through the out-projections (softmax weights sum to 1)
    saob = w["sa_out_b"] + w["sa_in_b"][2 * D:3 * D] @ w["sa_out_w"]
    caob = w["ca_out_b"] + w["ca_v_b"] @ w["ca_out_w"]

    bias = np.zeros((P, NBC), f32)
    for nm, src in [("qb", w["sa_in_b"][0:D]), ("kb", w["sa_in_b"][D:2 * D]),
                    ("saob", saob), ("caqb", w["ca_q_b"]),
                    ("cakb", w["ca_k_b"]), ("caob", caob),
                    ("ffb1", w["ff_b1"]), ("ffb2", w["ff_b2"]),
                    ("ln1g", w["ln1_g"]), ("ln1b", w["ln1_b"]),
                    ("ln2g", w["ln2_g"]), ("ln2b", w["ln2_b"]),
                    ("ln3g", w["ln3_g"]), ("ln3b", w["ln3_b"])]:
        c0, n = _BC[nm]
        bias[:, c0:c0 + n] = _bcol(src)

    wt = {
        "w_qkv": _pt(w["sa_in_w"]).astype(f8),
        "w_sao": _pt(w["sa_out_w"]).astype(f8),
        "w_caq": _pt(w["ca_q_w"]).astype(bf),
        "w_cak": _pt(w["ca_k_w"]).astype(f8),
        "w_cav": _pt(w["ca_v_w"]).astype(f8),
        "w_cao": _pt(w["ca_out_w"]).astype(f8),
        "w_ff1": _pt(w["ff_w1"]).astype(bf),
        "w_ff2": _pt(w["ff_w2"]).astype(bf),
        "bias": bias,
    }

    in_maps = []
    for c in range(NCORES):
        b, ch = c // 4, c % 4
        q0 = CH * ch
        # rotate tokens so the own chunk sits at columns 0:CH
        xtb = np.roll(_pt(x[b].T), -q0, axis=2)    # [128, EJ, S] f32
        m = {names[k]: v for k, v in wt.items()}
        m[names["xt8"]] = xtb.astype(f8)
        m[names["xt_f32"]] = np.ascontiguousarray(xtb[:, :, 0:CH]).astype(bf)
        m[names["yt"]] = _pt(y[b].T).astype(f8)
        in_maps.append(m)
    return in_maps


def assemble(results, out_name):
    out = np.zeros((B, S, D), np.float32)
    for c in range(NCORES):
        b, ch = c // 4, c % 4
        arr = np.asarray(results[c][out_name])     # [128, EJ, CH]
        out[b, CH * ch:CH * (ch + 1), :] = (
            arr.transpose(1, 0, 2).reshape(D, CH).T)
    return out


def run(inputs, **spmd_kwargs):
    nc, names, out_name = _compiled()
    in_maps = make_in_maps(inputs, names)
    res = run_bass_kernel_spmd(nc, in_maps, core_ids=list(range(NCORES)),
                               **spmd_kwargs)
    return assemble(res.results, out_name), res


def kernel(**inputs):
    out, _ = run(inputs)
    return out
